# revision 1
# baseline (speedup 1.0000x reference)
"""Trainium2 Bass kernel for the CMLIF (masked LIF over conv-mask) module.

Math being implemented:
    mask = (sigmoid(conv2d(ones) + b) > 0.5)            # batch-independent
    u_0 = 0
    u_{t+1} = 0.5 * u_t * (u_t <= 1) + x_t              # leaky integrate+reset
    o_t = (u_{t+1} > 1) * mask

Device trick: substitute q_t = 2^t * u_t (power-of-2 scaling is exact in
fp32).  Then
    q_{t+1} = q_t * (q_t <= 2^t) + 2^{t+1} * x_t
    o_t     = (q_{t+1} > 2^{t+1}) * mask
The leak multiplier disappears: the reset+leak is one fused DVE
scalar_tensor_tensor (q <= thr) * q, followed by one tensor_tensor add of
the host-prescaled x (exact).  This is bit-exact vs the reference.

The output compare runs on the otherwise-idle ScalarE (ACT) as
Sign(q - 2^(t+1)), the UNMASKED spike in {-1,0,1}; the batch-independent
conv mask is applied on the host during decode (spike = (code == 1) &
mask), so the device does zero mask work.  Output is int8 to cut HBM
write traffic 4x.

Sharding: data-parallel over batch across 8 NeuronCores; each core runs
the full 5-step recurrence on bs/8 images.  No cross-core communication.
"""

import numpy as np

TIME_STEP = 5
N_CORES = 8

LAST_RESULTS = None

_NC_CACHE = {}


def _import_concourse():
    try:
        import concourse.bass  # noqa: F401
    except ImportError:
        import sys

        for p in ("/opt/trn_rl_repo", "/root/.axon_site/_ro/trn_rl_repo"):
            if p not in sys.path:
                sys.path.append(p)
    import concourse.bacc as bacc
    import concourse.mybir as mybir
    from concourse.tile import TileContext
    from concourse.bass_utils import run_bass_kernel_spmd

    return bacc, mybir, TileContext, run_bass_kernel_spmd


def build_nc(
    B_l,
    C,
    HW,
    G,
    H,
    u_bufs=4,
    x_bufs=2,
    o_bufs=3,
    repeat=1,
    store_eng="gpsimd",
    split_loads=False,
):
    """Build the per-core Bass program.  (G is fixed at 1 in this layout.)

    DRAM layout (per core; image-major so timesteps are contiguous):
      x     [B_l, T, C, HW]  f32  -- host-prescaled by 2^(t+1); per image
                                     the 5 step-frames are contiguous, so
                                     steps (1,2) and (3,4) load as single
                                     4 MB DMAs.
      o     [B_l, T, C, HW]  int8 -- unmasked spike codes Sign(q-2^(t+1))
                                     in {-1,0,1}; host decodes
                                     spike = (code == 1) & mask.  All 5
                                     steps of an image store as one
                                     2.5 MB DMA.

    Engine split: DVE runs the recurrence (fused reset STT + add); ACT
    (ScalarE) computes the spike Sign; SP issues loads; Pool issues the
    coalesced stores over SWDGE.  (Pool compute is unusable: measured
    ~20x below line rate for bulk tensor_scalar on this stack.)
    """
    bacc, mybir, TileContext, _ = _import_concourse()
    f32, i8 = mybir.dt.float32, mybir.dt.int8
    Alu = mybir.AluOpType
    T = TIME_STEP
    assert G == 1
    W = HW
    Wd = HW // H

    nc = bacc.Bacc()
    xs = nc.declare_dram_parameter("x", [B_l, T, C, HW], f32, isOutput=False)
    oo = nc.declare_dram_parameter("o", [B_l, T, C, HW], i8, isOutput=True)

    with TileContext(nc) as tc:
        with (
            tc.tile_pool(name="const", bufs=1) as cpool,
            tc.tile_pool(name="u", bufs=u_bufs) as upool,
            tc.tile_pool(name="xt", bufs=x_bufs) as xpool,
            tc.tile_pool(name="ot", bufs=o_bufs) as opool,
        ):
            # bias columns -2^(t+1) for the ACT Sign (tiny, built on-chip)
            nbias = cpool.tile([C, T], f32)
            for t in range(T):
                nc.vector.memset(nbias[:, t : t + 1], -float(2 ** (t + 1)))

            for g in [g for _ in range(repeat) for g in range(B_l)]:
                u = upool.tile([C, W], f32, tag="u")
                # q_1 = 2*x_0 (host already scaled x_0 by 2)
                nc.sync.dma_start(out=u[:], in_=xs[g, 0])
                u3 = u[:].rearrange("c (h w) -> c h w", h=H)
                osx = opool.tile([C, T * W], i8, tag="osx")
                for t in range(T):
                    sc = float(2 ** (t + 1))
                    if t > 0:
                        if t % 2 == 1:
                            # steps (1,2) / (3,4) arrive as one 4 MB DMA
                            xp = xpool.tile([C, 2 * W], f32, tag="xp")
                            ld = nc.scalar if (split_loads and t == 3) else nc.sync
                            ld.dma_start(
                                out=xp[:].rearrange("c (t f) -> c t f", t=2),
                                in_=xs[g, t : t + 2].rearrange("t c f -> c t f"),
                            )
                        xt = xp[:, ((t - 1) % 2) * W : ((t - 1) % 2 + 1) * W]
                        # q~ = (q <= 2^t) * q
                        nc.vector.scalar_tensor_tensor(
                            u[:], u[:], float(2**t), u[:], Alu.is_le, Alu.mult
                        )
                        # q += 2^(t+1) * x_t
                        nc.vector.tensor_tensor(u[:], u[:], xt, Alu.add)
                    ot = osx[:, t * W : (t + 1) * W]
                    # unmasked spike on ACT: Sign(q - 2^(t+1)) in {-1,0,1};
                    # the conv mask is applied on the host during decode
                    nc.scalar.sign(ot, u[:], nbias[:, t : t + 1])
                # one 2.5 MB store for the whole image
                getattr(nc, store_eng).dma_start(
                    out=oo[g].rearrange("t c f -> c t f"),
                    in_=osx[:].rearrange("c (t f) -> c t f", t=TIME_STEP),
                )
    nc.compile()
    return nc


def compute_mask(conv_w, conv_b, H, W):
    """mask[c,h,w] = sigmoid(conv2d(ones)+b)[c,h,w] > 0.5  ==  z > 0.

    conv(ones) only depends on how much of the 3x3 kernel window is in
    bounds, so z = sum over valid (kh,kw) of s[c,kh,kw] + b[c], with
    s = conv_w.sum(axis=1).  Computed in f64 for a stable sign.
    """
    C = conv_w.shape[0]
    s = conv_w.astype(np.float64).sum(axis=1)  # [C,3,3]
    VH = np.zeros((H, 3))
    VW = np.zeros((W, 3))
    for k in range(3):
        VH[max(0, 1 - k) : min(H, H + 1 - k), k] = 1.0
        VW[max(0, 1 - k) : min(W, W + 1 - k), k] = 1.0
    z = np.einsum("ckl,hk,wl->chw", s, VH, VW) + conv_b.astype(np.float64)[:, None, None]
    return (z > 0).astype(np.float32).reshape(C, H * W)


def mask_aux(mask2d, H, Wd):
    """Threshold encodings of the mask.

    nthv [C,T]: -2^(t+1) where interior mask is 1 else -1e33 (ACT bias).
    thbr [C, 2*Wd]: border rows (h=0, h=H-1), 1.0 where mask else 1e30.
    thbc [C, H*2]:  border cols (w=0, w=Wd-1), same encoding.
    """
    C = mask2d.shape[0]
    m3 = mask2d.reshape(C, H, Wd)
    interior = m3[:, H // 2, Wd // 2]
    scales = (2.0 ** np.arange(1, TIME_STEP + 1)).astype(np.float32)
    nthv = np.where(
        interior[:, None] > 0, -scales[None, :], np.float32(-1e33)
    ).astype(np.float32)
    th3 = np.where(m3 > 0, np.float32(1.0), np.float32(1e30))
    rows = th3[:, [0, H - 1], :]  # [C, 2, Wd]
    cols = th3[:, :, [0, Wd - 1]]  # [C, H, 2]
    thbr = np.ascontiguousarray(rows.reshape(C, -1)).astype(np.float32)
    thbc = np.ascontiguousarray(cols.reshape(C, -1)).astype(np.float32)
    return nthv, thbr, thbc


def make_in_maps(x, conv_w, conv_b):
    """Per-core input dicts in the device layout, plus geometry."""
    T = TIME_STEP
    n, C, H, Wd = x.shape
    bs = n // T
    HW = H * Wd
    assert bs % N_CORES == 0, (bs, N_CORES)
    B_l = bs // N_CORES

    mask2d = compute_mask(conv_w, conv_b, H, Wd)

    # 2^(t+1) scaling, exact in fp32
    scales = (2.0 ** np.arange(1, T + 1)).astype(np.float32)
    x5 = x.reshape(T, bs, C, HW)
    in_maps = []
    for k in range(N_CORES):
        b0 = k * B_l
        # [B_l, T, C, HW] image-major, scaled; ufunc output is C-contiguous
        xc = x5[:, b0 : b0 + B_l].transpose(1, 0, 2, 3) * scales[None, :, None, None]
        in_maps.append({"x": xc})
    return in_maps, (B_l, C, HW, H, bs), mask2d


def kernel(x, conv_w, conv_b):
    global LAST_RESULTS
    _, _, _, run_bass_kernel_spmd = _import_concourse()

    T = TIME_STEP
    n, C, H, Wd = x.shape
    HW = H * Wd

    in_maps, (B_l, C, HW, H, bs), mask2d = make_in_maps(x, conv_w, conv_b)

    key = (B_l, C, HW, 1, H)
    if key not in _NC_CACHE:
        _NC_CACHE[key] = build_nc(*key)
    nc = _NC_CACHE[key]

    res = run_bass_kernel_spmd(nc, in_maps, list(range(N_CORES)))
    LAST_RESULTS = res

    # decode: device emits unmasked Sign(q - 2^(t+1)) codes {-1,0,1};
    # spike = (code == 1), then the conv mask is applied here
    mb = mask2d > 0  # [C, HW] bool
    out = np.empty((T, bs, C, HW), np.float32)
    for k in range(N_CORES):
        b0 = k * B_l
        ok = (res.results[k]["o"] == 1) & mb[None, None]  # [B_l,T,C,HW]
        out[:, b0 : b0 + B_l] = ok.transpose(1, 0, 2, 3)
    return out.reshape(n, C, H, Wd)



# revision 3
# speedup vs baseline: 18.4894x; 18.4894x over previous
"""Trainium2 Bass kernel for the CMLIF (masked LIF over conv-mask) module, v2.

Math:
    mask = (sigmoid(conv2d(ones) + b) > 0.5)            # batch-independent
    u_0 = 0
    u_{t+1} = 0.5 * u_t * (u_t <= 1) + x_t              # leaky integrate+reset
    o_t = (u_{t+1} > 1) * mask

Device design (per core, data-parallel over batch):
  * One fused custom DVE op per step:  u' = (u <= 1) * u * 0.5 + x
    (one 1x-rate instruction instead of STT+TT; bit-exact vs fp32 jax).
  * ScalarE (ACT) computes the unmasked spike Sign(u - 1) in {-1,0,1}
    into bf16 sign planes.
  * TensorE packs the 5 sign planes into one int8 code per element via
    PSUM-accumulated diagonal matmuls: code = sum_t 2^t * sign_t in
    [-31, 31]; ACT evicts PSUM -> int8 SBUF; one 0.5 MB store per image.
    Write traffic is 20x less than f32 output (1 byte per 5 timesteps).
  * The batch-independent conv mask is applied on the host during decode:
    bits = (code + 31) >> 1;  spike_t = ((bits >> t) & 1) & mask.

Sharding: batch across 8 NeuronCores, no cross-core communication.
"""

import numpy as np

TIME_STEP = 5
N_CORES = 8

LAST_RESULTS = None

_NC_CACHE = {}
_LIF_OP = None


def _import_concourse():
    try:
        import concourse.bass  # noqa: F401
    except ImportError:
        import sys

        for p in ("/opt/trn_rl_repo", "/root/.axon_site/_ro/trn_rl_repo"):
            if p not in sys.path:
                sys.path.append(p)
    import concourse.bacc as bacc
    import concourse.mybir as mybir
    from concourse.tile import TileContext
    from concourse.bass_utils import run_bass_kernel_spmd

    return bacc, mybir, TileContext, run_bass_kernel_spmd


def _lif_op():
    """Register (once) the fused LIF-step custom DVE op:
    out = (in0 <= s0) * in0 * s1 + in1, one DVE instruction at 1x rate."""
    global _LIF_OP
    if _LIF_OP is not None:
        return _LIF_OP
    _import_concourse()
    import concourse.dve_ops as dops
    from concourse.dve_spec import C0, C1, Spec, Src0, Src1, lower
    from concourse.dve_uop import DveOpSpec

    name = "LIF_STEP_ANT"
    for op in dops.OPS:
        if op.name == name:
            _LIF_OP = op
            return op
    spec = Spec(
        body=(Src0 <= C0) * Src0 * C1 + Src1,
        reference=lambda in0, in1, s0, s1, imm2: (
            np.where(in0 <= s0, in0, np.float32(0.0)).astype(np.float32)
            * np.float32(s1)
            + in1
        ).astype(np.float32),
    )
    row = dops._CUSTOM_DVE_ROW_BASE + len(dops.OPS)
    shas = {
        ver: DveOpSpec(
            name=name, opcode=row, uops=lower(spec, ver=ver), rd1_en=True
        ).sha(ver)
        for ver in ("v3", "v4")
    }
    op = dops.DveOp(name, spec, subdim=False, uops_sha=shas)
    dops.OPS.append(op)
    dops.CUSTOM_DVE_SPECS[name] = spec
    dops._SUB_OPCODE_FOR_NAME[name] = row
    _LIF_OP = op
    return op


def build_nc(B_l, C, HW, G, H, repeat=1, mode="pack"):
    """Per-core Bass program.

    DRAM (per core; image-major so each [C, HW] frame is one contiguous
    2 MB DMA):
      x  [B_l, T, C, HW] f32
      pw [C, T*128] bf16      pack weights: pw[:, t*128:(t+1)*128] = 2^t * I
      o  pack:   [B_l, C, HW]    int8 packed codes sum_t 2^t*sign_t
         planes: [B_l, T, C, HW] int8 sign codes {-1,0,1}

    Engines: DVE = fused recurrence (1 op/step); ACT = Sign + PSUM evict;
    PE = diagonal pack matmuls; SP = loads (HWDGE); Pool = stores (SWDGE).
    """
    bacc, mybir, TileContext, _ = _import_concourse()
    f32, i8, bf16 = mybir.dt.float32, mybir.dt.int8, mybir.dt.bfloat16
    T = TIME_STEP
    W = HW
    lif = _lif_op()

    nc = bacc.Bacc()
    xs = nc.declare_dram_parameter("x", [B_l, T, C, HW], f32, isOutput=False)
    if mode == "pack":
        pw = nc.declare_dram_parameter("pw", [C, T * 128], bf16, isOutput=False)
        oo = nc.declare_dram_parameter("o", [B_l, C, HW], i8, isOutput=True)
    else:
        oo = nc.declare_dram_parameter("o", [B_l, T, C, HW], i8, isOutput=True)

    with TileContext(nc) as tc:
        with (
            tc.tile_pool(name="const", bufs=1) as cpool,
            tc.tile_pool(name="u0", bufs=2) as u0pool,
            tc.tile_pool(name="xt", bufs=3) as xtpool,
            tc.tile_pool(name="u", bufs=2) as upool,
            tc.tile_pool(name="pl", bufs=2) as plpool,
            tc.tile_pool(name="ot", bufs=2) as opool,
            tc.tile_pool(name="ps", bufs=1, space="PSUM") as pspool,
        ):
            nbias = cpool.tile([C, 1], f32)
            nc.vector.memset(nbias[:], -1.0)
            if mode == "pack":
                wt = cpool.tile([C, T * 128], bf16)
                nc.sync.dma_start(out=wt[:], in_=pw[:])

            for g in [g for _ in range(repeat) for g in range(B_l)]:
                u1 = u0pool.tile([C, W], f32, tag="u0")
                nc.sync.dma_start(out=u1[:], in_=xs[g, 0])
                if mode == "pack":
                    ps = pspool.tile([C, W], f32, tag="ps")
                else:
                    osx = opool.tile([C, T * W], i8, tag="ot")
                ucur = u1
                for t in range(T):
                    # unmasked spike of step t: Sign(u_{t+1} - 1)
                    if mode == "pack":
                        pl = plpool.tile([C, W], bf16, tag="pl")
                        nc.scalar.sign(pl[:], ucur[:], nbias[:])
                        for j in range(W // 512):
                            s = slice(j * 512, (j + 1) * 512)
                            nc.tensor.matmul(
                                ps[:, s],
                                wt[:, t * 128 : (t + 1) * 128],
                                pl[:, s],
                                start=(t == 0),
                                stop=(t == T - 1),
                            )
                    else:
                        nc.scalar.sign(osx[:, t * W : (t + 1) * W], ucur[:], nbias[:])
                    if t < T - 1:
                        if t % 2 == 0:
                            # frames (t+1, t+2) arrive as one 4 MB DMA
                            xp = xtpool.tile([C, 2 * W], f32, tag="xt")
                            nc.sync.dma_start(
                                out=xp[:].rearrange("c (t f) -> c t f", t=2),
                                in_=xs[g, t + 1 : t + 3].rearrange("t c f -> c t f"),
                            )
                        xt = xp[:, (t % 2) * W : (t % 2 + 1) * W]
                        un = upool.tile([C, W], f32, tag="u")
                        nc.vector._custom_dve(
                            lif, out=un[:], in0=ucur[:], in1=xt, s0=1.0, s1=0.5
                        )
                        ucur = un
                if mode == "pack":
                    ot = opool.tile([C, W], i8, tag="ot")
                    nc.scalar.copy(ot[:], ps[:])
                    nc.gpsimd.dma_start(out=oo[g], in_=ot[:])
                else:
                    nc.gpsimd.dma_start(
                        out=oo[g].rearrange("t c f -> c t f"),
                        in_=osx[:].rearrange("c (t f) -> c t f", t=T),
                    )
    nc.compile()
    return nc


def compute_mask(conv_w, conv_b, H, W):
    """mask[c,h,w] = sigmoid(conv2d(ones)+b)[c,h,w] > 0.5  ==  z > 0.

    conv(ones) only depends on how much of the 3x3 kernel window is in
    bounds, so z = sum over valid (kh,kw) of s[c,kh,kw] + b[c], with
    s = conv_w.sum(axis=1).  Computed in f64 for a stable sign.
    """
    C = conv_w.shape[0]
    s = conv_w.astype(np.float64).sum(axis=1)  # [C,3,3]
    VH = np.zeros((H, 3))
    VW = np.zeros((W, 3))
    for k in range(3):
        VH[max(0, 1 - k) : min(H, H + 1 - k), k] = 1.0
        VW[max(0, 1 - k) : min(W, W + 1 - k), k] = 1.0
    z = np.einsum("ckl,hk,wl->chw", s, VH, VW) + conv_b.astype(np.float64)[:, None, None]
    return (z > 0).astype(np.float32).reshape(C, H * W)


def _pack_weights(C):
    """pw[c, t*128 + k] = 2^t * (c == k), bf16."""
    _, mybir, _, _ = _import_concourse()
    bf16 = mybir.dt.np(mybir.dt.bfloat16)
    T = TIME_STEP
    pw = np.zeros((C, T * 128), np.float32)
    eye = np.eye(C, 128, dtype=np.float32)
    for t in range(T):
        pw[:, t * 128 : (t + 1) * 128] = eye * float(2**t)
    return pw.astype(bf16)


def make_in_maps(x, conv_w, conv_b, mode="pack"):
    """Per-core input dicts in the device layout, plus geometry."""
    T = TIME_STEP
    n, C, H, Wd = x.shape
    bs = n // T
    HW = H * Wd
    assert bs % N_CORES == 0, (bs, N_CORES)
    B_l = bs // N_CORES

    mask2d = compute_mask(conv_w, conv_b, H, Wd)

    x5 = x.reshape(T, bs, C, HW)
    pw = _pack_weights(C) if mode == "pack" else None
    in_maps = []
    for k in range(N_CORES):
        b0 = k * B_l
        # [B_l, T, C, HW] image-major; ascontiguousarray for clean DMA
        xc = np.ascontiguousarray(x5[:, b0 : b0 + B_l].transpose(1, 0, 2, 3))
        m = {"x": xc}
        if mode == "pack":
            m["pw"] = pw
        in_maps.append(m)
    return in_maps, (B_l, C, HW, H, bs), mask2d


MODE = "pack"


def kernel(x, conv_w, conv_b):
    global LAST_RESULTS
    _, _, _, run_bass_kernel_spmd = _import_concourse()

    T = TIME_STEP
    n, C, H, Wd = x.shape
    HW = H * Wd

    in_maps, (B_l, C, HW, H, bs), mask2d = make_in_maps(x, conv_w, conv_b, mode=MODE)

    key = (B_l, C, HW, 1, H, MODE)
    if key not in _NC_CACHE:
        _NC_CACHE[key] = build_nc(B_l, C, HW, 1, H, mode=MODE)
    nc = _NC_CACHE[key]

    res = run_bass_kernel_spmd(nc, in_maps, list(range(N_CORES)))
    LAST_RESULTS = res

    mb = mask2d > 0  # [C, HW] bool
    out = np.empty((T, bs, C, HW), np.float32)
    for k in range(N_CORES):
        b0 = k * B_l
        code = res.results[k]["o"]
        if MODE == "pack":
            # code = sum_t 2^t * sign_t in [-31, 31]
            bits = ((code.astype(np.int16) + 31) >> 1).astype(np.uint8)
            for t in range(T):
                ok = (((bits >> t) & 1) > 0) & mb[None]  # [B_l, C, HW]
                out[t, b0 : b0 + B_l] = ok
        else:
            ok = (code == 1) & mb[None, None]  # [B_l, T, C, HW]
            out[:, b0 : b0 + B_l] = ok.transpose(1, 0, 2, 3)
    return out.reshape(n, C, H, Wd)
